# revision 1
# baseline (speedup 1.0000x reference)
"""Trainium2 Bass kernel for nn_CapacityTestMemory (scatter_memory).

reference computation:
    memory  = round-robin circular buffer of enc_hidden rows   (B, M, H)
    q       = query_hidden @ q_w + q_b                         (B, H)
    k       = memory @ k_w + k_b                               (B, M, H)
    raw     = einsum('bh,bmh->bm', q, k) / sqrt(H)             (B, M)
    attn    = softmax over top-8 of raw, 0 elsewhere           (B, M)
    out     = (einsum('bm,bmh->bh', attn, memory) + query) @ out_w + out_b

Key simplifications used here (exact, not approximations):
  *  raw[b,m] = memory[b,m,:] . (k_w @ q[b]) / sqrt(H)  +  q[b].k_b/sqrt(H).
     The k_b term is a per-batch constant added to every slot's score; a
     constant shift changes neither the top-k selection nor the softmax
     probabilities, so it is dropped.
  *  The final output depends on attn only through the weighted row sum, so
     the slot ordering of the circular buffer is irrelevant.  The set of live
     memory rows is a single contiguous range of enc_hidden positions
     [max(0, L-M), L) with L = min(2*num_pairs, T-3), so the "scatter" gather
     collapses to one contiguous DMA per batch.

Sharding: pure data parallel, batch 32 -> 4 batches per core x 8 cores.
"""

import math
from contextlib import ExitStack

import numpy as np

import concourse.bacc as bacc
import concourse.mybir as mybir
from concourse.bass import IndirectOffsetOnAxis
from concourse.masks import make_identity
from concourse.tile import TileContext
from concourse.bass_utils import run_bass_kernel_spmd

B, T, H = 32, 4096, 512
M = 2048            # memory slots
TOPK = 8
VOCAB = 128
NCORES = 8
BP = B // NCORES    # batches per core
G = M // 128        # slot groups of 128
HC = H // 128       # h chunks of 128
F32 = mybir.dt.float32
I32 = mybir.dt.int32
U32 = mybir.dt.uint32

_CACHE = {}


def _build_kernel(reps=1, elayout="gp", loop_reps=1, peg=5):
    nc = bacc.Bacc("TRN2", target_bir_lowering=False, debug=False, num_devices=NCORES)

    enc = nc.dram_tensor("enc", [BP, M, H], F32, kind="ExternalInput")
    query = nc.dram_tensor("query", [BP, H], F32, kind="ExternalInput")
    q_w = nc.dram_tensor("q_w", [H, H], F32, kind="ExternalInput")
    q_b = nc.dram_tensor("q_b", [H], F32, kind="ExternalInput")
    k_w = nc.dram_tensor("k_w", [H, H], F32, kind="ExternalInput")
    out_w = nc.dram_tensor("out_w", [H, VOCAB], F32, kind="ExternalInput")
    out_b = nc.dram_tensor("out_b", [VOCAB], F32, kind="ExternalInput")
    logits = nc.dram_tensor("logits", [BP, VOCAB], F32, kind="ExternalOutput")

    with TileContext(nc) as tc, ExitStack() as ctx:
        cpool = ctx.enter_context(tc.tile_pool(name="const", bufs=1))
        wpool = ctx.enter_context(tc.tile_pool(name="weights", bufs=1))
        epool = ctx.enter_context(tc.tile_pool(name="enc", bufs=2))
        spool = ctx.enter_context(tc.tile_pool(name="scratch", bufs=1))
        qpool = ctx.enter_context(tc.tile_pool(name="qkb", bufs=1))
        rpool = ctx.enter_context(tc.tile_pool(name="rep", bufs=2))
        pp_big = ctx.enter_context(tc.tile_pool(name="ppbig", bufs=1, space="PSUM"))
        pp_sm = ctx.enter_context(tc.tile_pool(name="ppsm", bufs=1, space="PSUM"))
        pp_acc = ctx.enter_context(tc.tile_pool(name="ppacc", bufs=1, space="PSUM"))
        pp_et = ctx.enter_context(tc.tile_pool(name="ppet", bufs=2, space="PSUM"))
        pp_srow = ctx.enter_context(tc.tile_pool(name="ppsrow", bufs=1, space="PSUM"))

        # ---- constants -------------------------------------------------
        ident128 = cpool.tile([128, 128], F32)
        make_identity(nc, ident128[:])
        ident4 = cpool.tile([4, 4], F32)
        make_identity(nc, ident4[:])
        ones1_bp = cpool.tile([1, BP], F32)
        nc.vector.memset(ones1_bp[:], 1.0)
        # block-diagonal ones: blk[k, m] = 1 iff k // TOPK == m
        blk_dram = nc.inline_tensor(
            np.kron(np.eye(BP), np.ones((TOPK, 1))).astype(np.float32), name="blk"
        )
        blk = cpool.tile([BP * TOPK, BP], F32)
        nc.sync.dma_start(out=blk[:], in_=blk_dram[:])
        # per-batch flat-row offset b*M (as float, exact for these magnitudes)
        boff_dram = nc.inline_tensor(
            (np.arange(BP, dtype=np.float32) * M)[:, None], name="boff"
        )
        boff = cpool.tile([BP, 1], F32)
        nc.sync.dma_start(out=boff[:], in_=boff_dram[:])

        # ---- weight / small input loads --------------------------------
        query_sb = wpool.tile([BP, H], F32)
        nc.sync.dma_start(out=query_sb[:], in_=query[:])
        qw_sb = wpool.tile([128, HC, H], F32)
        nc.sync.dma_start(out=qw_sb[:], in_=q_w[:].rearrange("(c p) h -> p c h", p=128))
        kw_sb = wpool.tile([128, HC, H], F32)
        nc.sync.dma_start(out=kw_sb[:], in_=k_w[:].rearrange("(c p) h -> p c h", p=128))
        ow_sb = wpool.tile([128, HC, VOCAB], F32)
        nc.sync.dma_start(out=ow_sb[:], in_=out_w[:].rearrange("(c p) v -> p c v", p=128))
        qb_sb = wpool.tile([1, H], F32)
        nc.sync.dma_start(out=qb_sb[:], in_=q_b[None, :])
        ob_sb = wpool.tile([1, VOCAB], F32)
        nc.sync.dma_start(out=ob_sb[:], in_=out_b[None, :])

        # ---- prologue: qk[b] = (k_w @ (q_w^T query[b] + q_b)) / sqrt(H) --
        # query^T: [BP, H] -> HC chunks of [128, BP]
        qT_ps = pp_sm.tile([128, HC * BP], F32, tag="tps")
        for c in range(HC):
            nc.tensor.transpose(
                out=qT_ps[:, c * BP:(c + 1) * BP],
                in_=query_sb[:, c * 128:(c + 1) * 128],
                identity=ident4[:],
            )
        qT_sb = wpool.tile([128, HC * BP], F32)
        nc.scalar.copy(out=qT_sb[:], in_=qT_ps[:])

        # qa = query @ q_w + q_b  (accumulated in PSUM, bias via ones matmul)
        qa_ps = pp_acc.tile([BP, H], F32, tag="acc")
        nc.tensor.matmul(out=qa_ps[:], lhsT=ones1_bp[:], rhs=qb_sb[:], start=True, stop=False)
        for c in range(HC):
            nc.tensor.matmul(
                out=qa_ps[:],
                lhsT=qT_sb[:, c * BP:(c + 1) * BP],
                rhs=qw_sb[:, c, :],
                start=False,
                stop=(c == HC - 1),
            )
        qa_sb = wpool.tile([BP, H], F32)
        nc.scalar.copy(out=qa_sb[:], in_=qa_ps[:])

        # qa^T chunks
        qaT_ps = pp_sm.tile([128, HC * BP], F32, tag="tps")
        for c in range(HC):
            nc.tensor.transpose(
                out=qaT_ps[:, c * BP:(c + 1) * BP],
                in_=qa_sb[:, c * 128:(c + 1) * 128],
                identity=ident4[:],
            )
        qaT_sb = wpool.tile([128, HC * BP], F32)
        nc.scalar.copy(out=qaT_sb[:], in_=qaT_ps[:])

        # k_w^T (16 PE transposes of 128x128 blocks)
        kwT_sb = wpool.tile([128, HC, H], F32)
        for r in range(HC):
            for c in range(HC):
                t_ps = pp_sm.tile([128, 128], F32, tag="tps")
                nc.tensor.transpose(
                    out=t_ps[:],
                    in_=kw_sb[:, r, c * 128:(c + 1) * 128],
                    identity=ident128[:],
                )
                nc.scalar.copy(out=kwT_sb[:, c, r * 128:(r + 1) * 128], in_=t_ps[:])

        # qk = qa @ k_w^T   (contraction over h' using qaT / kwT)
        qk_ps = pp_acc.tile([BP, H], F32, tag="acc")
        for c in range(HC):
            nc.tensor.matmul(
                out=qk_ps[:],
                lhsT=qaT_sb[:, c * BP:(c + 1) * BP],
                rhs=kwT_sb[:, c, :],
                start=(c == 0),
                stop=(c == HC - 1),
            )
        qk_rows = wpool.tile([BP, H], F32)
        nc.scalar.mul(out=qk_rows[:], in_=qk_ps[:], mul=1.0 / math.sqrt(H))

        # qk^T chunks (for PE-scored groups): qkT[:, c*BP+b] = qk[b, 128c:...]
        qkT_ps = pp_sm.tile([128, HC * BP], F32, tag="tps")
        for c in range(HC):
            nc.tensor.transpose(
                out=qkT_ps[:, c * BP:(c + 1) * BP],
                in_=qk_rows[:, c * 128:(c + 1) * 128],
                identity=ident4[:],
            )
        qkT_sb = wpool.tile([128, HC * BP], F32)
        nc.scalar.copy(out=qkT_sb[:], in_=qkT_ps[:])

        # broadcast each batch's qk row across 128 partitions (via DRAM
        # bounce: DMA supports partition-stride-0 broadcast from DRAM)
        qk_dram = nc.dram_tensor("qk_scratch", [BP, H], F32)
        nc.sync.dma_start(out=qk_dram[:], in_=qk_rows[:])
        qkb_sbs = []
        for b in range(BP):
            qkb_sb = qpool.tile([128, H], F32, tag=f"qkb{b}")
            nc.sync.dma_start(
                out=qkb_sb[:], in_=qk_dram[b][None, :].to_broadcast([128, H])
            )
            qkb_sbs.append(qkb_sb)

        # ---- main loop: scores for all slots ---------------------------
        import contextlib
        loop_cm = tc.For_i(0, loop_reps, 1) if loop_reps > 1 else contextlib.nullcontext()
        with loop_cm:
            for rep in range(reps):
                dg = G - peg  # groups scored on DVE; last peg groups go to PE
                scores_col = rpool.tile([128, BP * dg], F32, tag="scol")
                junk = rpool.tile([128, H], F32, tag="junk")
                scores_row = rpool.tile([BP, M], F32, tag="scores_row")
                for b in range(BP):
                    e_sb = epool.tile([128, G, H], F32, tag="e")
                    if elayout == "gp":
                        e_in = enc[b].rearrange("(g p) h -> p g h", p=128)
                    else:
                        e_in = enc[b].rearrange("(p g) h -> p g h", g=G)
                    nc.sync.dma_start(out=e_sb[:], in_=e_in)
                    for g in range(dg):
                        nc.vector.scalar_tensor_tensor(
                            out=junk[:],
                            in0=e_sb[:, g, :],
                            scalar=1.0,
                            in1=qkb_sbs[b][:],
                            op0=mybir.AluOpType.mult,
                            op1=mybir.AluOpType.mult,
                            accum_out=scores_col[:, b * dg + g: b * dg + g + 1],
                        )
                    if peg:
                        srow_ps = pp_srow.tile([1, peg * 128], F32, tag="srow")
                        for gi in range(peg):
                            g = dg + gi
                            et_ps = pp_et.tile([128, H], F32, tag="et")
                            for c in range(HC):
                                nc.tensor.transpose(
                                    out=et_ps[:, c * 128:(c + 1) * 128],
                                    in_=e_sb[:, g, c * 128:(c + 1) * 128],
                                    identity=ident128[:],
                                )
                            et_sb = rpool.tile([128, H], F32, tag="et_sb")
                            nc.scalar.copy(out=et_sb[:], in_=et_ps[:])
                            for c in range(HC):
                                nc.tensor.matmul(
                                    out=srow_ps[:, gi * 128:(gi + 1) * 128],
                                    lhsT=qkT_sb[:, c * BP + b: c * BP + b + 1],
                                    rhs=et_sb[:, c * 128:(c + 1) * 128],
                                    start=(c == 0),
                                    stop=(c == HC - 1),
                                )
                        srow_sb = rpool.tile([1, peg * 128], F32, tag="srow_sb")
                        nc.scalar.copy(out=srow_sb[:], in_=srow_ps[:])
                        nc.scalar.dma_start(
                            out=scores_row[b:b + 1, dg * 128:], in_=srow_sb[:]
                        )

                # ---- top-8 ------------------------------------------------------
                # transpose DVE-scored cols [128, BP*dg] -> [BP*dg, 128] -> rows
                sT_ps = pp_big.tile([BP * dg, 128], F32, tag="qkbps")
                nc.tensor.transpose(out=sT_ps[:], in_=scores_col[:], identity=ident128[:])
                sT_sb = rpool.tile([BP * dg, 128], F32, tag="sT_sb")
                nc.scalar.copy(out=sT_sb[:], in_=sT_ps[:])
                nc.scalar.dma_start(out=scores_row[:, :dg * 128], in_=sT_sb[:])

                vals = rpool.tile([BP, TOPK], F32, tag="vals")
                idx = rpool.tile([BP, TOPK], U32, tag="idx")
                nc.vector.max(out=vals[:], in_=scores_row[:])
                nc.vector.max_index(out=idx[:], in_max=vals[:], in_values=scores_row[:])

                # ---- softmax over the 8 values ----------------------------------
                neg_m = rpool.tile([BP, 1], F32, tag="neg_m")
                nc.scalar.mul(out=neg_m[:], in_=vals[:, 0:1], mul=-1.0)
                esb = rpool.tile([BP, TOPK], F32, tag="esb")
                nc.scalar.activation(
                    out=esb[:], in_=vals[:], func=mybir.ActivationFunctionType.Exp,
                    bias=neg_m[:, :1], scale=1.0,
                )
                zsum = rpool.tile([BP, 1], F32, tag="zsum")
                nc.vector.reduce_sum(out=zsum[:], in_=esb[:], axis=mybir.AxisListType.X)
                rz = rpool.tile([BP, 1], F32, tag="rz")
                nc.vector.reciprocal(out=rz[:], in_=zsum[:])
                probs = rpool.tile([BP, TOPK], F32, tag="probs")
                nc.vector.tensor_scalar_mul(probs[:], esb[:], rz[:, :1])

                # ---- gather the 8 winning rows per batch ------------------------
                if elayout == "pg":
                    # row j = g*128 + p maps to slot m = p*G + g
                    gi = rpool.tile([BP, TOPK], U32, tag="gi")
                    pi = rpool.tile([BP, TOPK], U32, tag="pi")
                    nc.vector.tensor_scalar(
                        out=gi[:], in0=idx[:], scalar1=7, scalar2=None,
                        op0=mybir.AluOpType.logical_shift_right,
                    )
                    nc.vector.tensor_scalar(
                        out=pi[:], in0=idx[:], scalar1=127, scalar2=None,
                        op0=mybir.AluOpType.bitwise_and,
                    )
                    nc.vector.tensor_scalar(
                        out=pi[:], in0=pi[:], scalar1=4, scalar2=None,
                        op0=mybir.AluOpType.logical_shift_left,
                    )
                    nc.vector.tensor_tensor(
                        out=idx[:], in0=pi[:], in1=gi[:], op=mybir.AluOpType.add
                    )
                idxf = rpool.tile([BP, TOPK], F32, tag="idxf")
                nc.vector.tensor_copy(idxf[:], idx[:])
                nc.vector.tensor_scalar_add(idxf[:], idxf[:], boff[:, :1])
                idx_flat = rpool.tile([BP, TOPK], I32, tag="idx_flat")
                nc.vector.tensor_copy(idx_flat[:], idxf[:])

                combo = rpool.tile([BP, TOPK, 2], F32, tag="combo")
                nc.vector.tensor_copy(combo[:, :, 0], probs[:])
                nc.vector.tensor_copy(combo[:, :, 1].bitcast(I32), idx_flat[:])
                combo_col = rpool.tile([BP * TOPK, 2], F32, tag="combo_col")
                nc.scalar.dma_start(out=combo_col[:], in_=combo[:])
                probs_col = combo_col[:, 0:1]
                idx_col = combo_col[:, 1:2].bitcast(I32)

                rows_sb = rpool.tile([BP * TOPK, H], F32, tag="rows_sb")
                nc.gpsimd.indirect_dma_start(
                    out=rows_sb[:],
                    out_offset=None,
                    in_=enc[:].rearrange("b m h -> (b m) h"),
                    in_offset=IndirectOffsetOnAxis(ap=idx_col, axis=0),
                )

                # ---- retrieved^T = rows^T @ blk;  xT = retT + queryT ------------
                nc.vector.tensor_scalar_mul(rows_sb[:], rows_sb[:], probs_col)
                retT_ps = pp_sm.tile([128, HC * BP], F32, tag="tps")
                for c in range(HC):
                    nc.tensor.matmul(
                        out=retT_ps[:, c * BP:(c + 1) * BP],
                        lhsT=rows_sb[:, c * 128:(c + 1) * 128],
                        rhs=blk[:],
                        start=True,
                        stop=True,
                    )
                xT_sb = rpool.tile([128, HC * BP], F32, tag="xT_sb")
                nc.vector.tensor_add(out=xT_sb[:], in0=retT_ps[:], in1=qT_sb[:])

                log_ps = pp_acc.tile([BP, VOCAB], F32, tag="acc")
                nc.tensor.matmul(out=log_ps[:], lhsT=ones1_bp[:], rhs=ob_sb[:], start=True, stop=False)
                for c in range(HC):
                    nc.tensor.matmul(
                        out=log_ps[:],
                        lhsT=xT_sb[:, c * BP:(c + 1) * BP],
                        rhs=ow_sb[:, c, :],
                        start=False,
                        stop=(c == HC - 1),
                    )
                log_sb = rpool.tile([BP, VOCAB], F32, tag="log_sb")
                nc.scalar.copy(out=log_sb[:], in_=log_ps[:])
                nc.sync.dma_start(out=logits[:], in_=log_sb[:])

    nc.compile()
    return nc


DEFAULT_ELAYOUT = "gp"


DEFAULT_PEG = 5


def get_nc(reps=1, elayout=None, loop_reps=1, peg=None):
    if elayout is None:
        elayout = DEFAULT_ELAYOUT
    if peg is None:
        peg = DEFAULT_PEG
    key = (reps, elayout, loop_reps, peg)
    if key not in _CACHE:
        _CACHE[key] = _build_kernel(reps, elayout, loop_reps, peg)
    return _CACHE[key]


def _prepare_in_maps(enc_hidden, query_hidden, num_pairs, q_w, q_b, k_w, out_w, out_b):
    L = min(2 * int(num_pairs), T - 3)
    n_valid = max(0, min(L, M))
    start = max(0, L - M)

    q_w = np.ascontiguousarray(q_w, dtype=np.float32)
    q_b = np.ascontiguousarray(q_b, dtype=np.float32)
    k_w = np.ascontiguousarray(k_w, dtype=np.float32)
    out_w = np.ascontiguousarray(out_w, dtype=np.float32)
    out_b = np.ascontiguousarray(out_b, dtype=np.float32)

    in_maps = []
    for core in range(NCORES):
        b0 = core * BP
        sl = np.asarray(enc_hidden[b0:b0 + BP, start:start + n_valid, :], dtype=np.float32)
        if n_valid < M:
            pad = np.zeros((BP, M, H), dtype=np.float32)
            pad[:, :n_valid, :] = sl
            sl = pad
        else:
            sl = np.ascontiguousarray(sl)
        in_maps.append({
            "enc": sl,
            "query": np.ascontiguousarray(query_hidden[b0:b0 + BP, :], dtype=np.float32),
            "q_w": q_w,
            "q_b": q_b,
            "k_w": k_w,
            "out_w": out_w,
            "out_b": out_b,
        })
    return in_maps


def kernel(enc_hidden, query_hidden, num_pairs, q_w, q_b, k_w, k_b, out_w, out_b,
           **run_kwargs):
    """Full-input entry point: shards across 8 NeuronCores, returns (B, VOCAB).

    k_b is accepted (to match the reference signature) but unused: it shifts
    every attention score by the same per-batch constant, which affects
    neither the top-k selection nor the softmax probabilities.
    """
    enc_hidden = np.asarray(enc_hidden)
    query_hidden = np.asarray(query_hidden)
    nc = get_nc()
    in_maps = _prepare_in_maps(
        enc_hidden, query_hidden, num_pairs, q_w, q_b, k_w, out_w, out_b
    )
    res = run_bass_kernel_spmd(nc, in_maps, core_ids=list(range(NCORES)), **run_kwargs)
    out = np.concatenate([res.results[c]["logits"] for c in range(NCORES)], axis=0)
    kernel.last_results = res
    return out



# revision 20
# speedup vs baseline: 1.6338x; 1.6338x over previous
"""Trainium2 Bass kernel for nn_CapacityTestMemory (scatter_memory).

reference computation:
    memory  = round-robin circular buffer of enc_hidden rows   (B, M, H)
    q       = query_hidden @ q_w + q_b                         (B, H)
    k       = memory @ k_w + k_b                               (B, M, H)
    raw     = einsum('bh,bmh->bm', q, k) / sqrt(H)             (B, M)
    attn    = softmax over top-8 of raw, 0 elsewhere           (B, M)
    out     = (einsum('bm,bmh->bh', attn, memory) + query) @ out_w + out_b

Exact simplifications:
  *  raw[b,m] = memory[b,m,:] . (k_w @ q[b]) / sqrt(H) + const(b); the
     constant (from k_b) shifts every slot equally -> dropped.
  *  The live memory rows are the contiguous enc positions [L-M, L),
     L = min(2*num_pairs, T-3): the circular buffer collapses to a slice.

v2 performance strategy:
  *  scoring runs on the TensorEngine: enc is uploaded host-transposed in
     fp8 (qk^T is the 128x4 stationary, enc^T chunks are the moving
     operand).  fp8 quarters the dominant HBM traffic; scoring accuracy
     only has to place the true top-8 inside a 32-candidate set (margin
     is ~8 sigma of the fp8 noise).
  *  candidates: per 512-slot window, MAX8 + FIND_INDEX8 on the f32
     score rows; 8 candidates/window x 4 windows = 32 per batch.
  *  exact rescore: the 32 candidate rows are gathered from a full-f32
     enc copy and re-scored on DVE against an f32 qk (f32 q_w/k_w
     prologue), so the final top-8 selection and softmax probabilities
     match the f32 reference to ~1e-6.

Sharding: pure data parallel, batch 32 -> 4 batches per core x 8 cores.
"""

import math
from contextlib import ExitStack

import numpy as np

import concourse.bacc as bacc
import concourse.mybir as mybir
from concourse.bass import IndirectOffsetOnAxis
from concourse.masks import make_identity
from concourse.tile import TileContext
from concourse.bass_utils import run_bass_kernel_spmd

B, T, H = 32, 4096, 512
M = 2048            # memory slots
TOPK = 8
VOCAB = 128
NCORES = 8
BP = B // NCORES    # batches per core
HC = H // 128       # h chunks of 128
NCHUNK = 4          # slot chunks (DMA + scoring granularity)
S = M // NCHUNK     # slots per chunk
NCAND = NCHUNK * 8  # candidates per batch
F32 = mybir.dt.float32
FP8 = mybir.dt.float8e4
I32 = mybir.dt.int32
U32 = mybir.dt.uint32

_CACHE = {}


def _build_kernel():
    nc = bacc.Bacc("TRN2", target_bir_lowering=False, debug=False, num_devices=NCORES)

    # host layouts (see _prepare_in_maps):
    #   encT8[j][32*b+hr][(hb, s)] = enc[b, j*S+s, 32*hb+hr]   (fp8)
    #     -> one matmul with a block-diagonal stationary scores all 4
    #        batches at once: 128-partition contraction = 4 batches x 32 h
    #   enc32 = f32 enc slice, natural [BP, M, H] (gather source only)
    #   qw32  = q_w   "(c p) h -> p (c h)"
    #   kwt32 = k_w^T "(c p) h -> p (c h)"
    #   oww   = out_w "(c p) v -> p (c v)"
    HB = H // 32        # 32-row h blocks in the batched contraction
    encT8 = nc.dram_tensor("encT8", [NCHUNK, 128, HB * S], FP8, kind="ExternalInput")
    enc32 = nc.dram_tensor("enc32", [BP, M, H], F32, kind="ExternalInput")
    query = nc.dram_tensor("query", [BP, H], F32, kind="ExternalInput")
    qw32 = nc.dram_tensor("qw32", [128, HC * H], F32, kind="ExternalInput")
    kwt32 = nc.dram_tensor("kwt32", [128, HC * H], F32, kind="ExternalInput")
    qb = nc.dram_tensor("qb", [H], F32, kind="ExternalInput")
    oww = nc.dram_tensor("oww", [128, HC * VOCAB], F32, kind="ExternalInput")
    ob = nc.dram_tensor("ob", [VOCAB], F32, kind="ExternalInput")
    logits = nc.dram_tensor("logits", [BP, VOCAB], F32, kind="ExternalOutput")

    with TileContext(nc) as tc, ExitStack() as ctx:
        cpool = ctx.enter_context(tc.tile_pool(name="const", bufs=1))
        wpool = ctx.enter_context(tc.tile_pool(name="weights", bufs=1))
        epool = ctx.enter_context(tc.tile_pool(name="enc", bufs=3))
        rpool = ctx.enter_context(tc.tile_pool(name="rep", bufs=1))
        pp_sm = ctx.enter_context(tc.tile_pool(name="ppsm", bufs=1, space="PSUM"))
        pp_acc = ctx.enter_context(tc.tile_pool(name="ppacc", bufs=1, space="PSUM"))
        pp_b = ctx.enter_context(tc.tile_pool(name="ppb", bufs=1, space="PSUM"))
        pp_s = ctx.enter_context(tc.tile_pool(name="pps", bufs=3, space="PSUM"))

        # ---- big DMAs on the sync ring: weights first, then enc chunks ---
        qw_sb = wpool.tile([128, HC * H], F32)
        nc.sync.dma_start(out=qw_sb[:], in_=qw32[:])
        kwt_sb = wpool.tile([128, HC * H], F32)
        nc.sync.dma_start(out=kwt_sb[:], in_=kwt32[:])
        e_sbs = []
        for j in range(NCHUNK):
            e_sb = epool.tile([128, BP * HC * S], FP8, tag="e")
            nc.sync.dma_start(out=e_sb[:], in_=encT8[j])
            e_sbs.append(e_sb)

        # ---- constants / small loads (scalar=ACT HWDGE ring) -------------
        ident4 = cpool.tile([4, 4], F32)
        make_identity(nc, ident4[:])
        ones1_bp = cpool.tile([1, BP], F32)
        nc.vector.memset(ones1_bp[:], 1.0)
        # sel128[b', b*NCAND_? ] -- broadcast selector: row b' one at cols b'*32..
        sel_dram = nc.inline_tensor(
            np.kron(np.eye(BP), np.ones((1, NCAND))).astype(np.float32), name="sel"
        )
        sel128 = cpool.tile([BP, BP * NCAND], F32)
        nc.scalar.dma_start(out=sel128[:], in_=sel_dram[:])
        # blk32[b*NCAND+k, b'] = (b == b')
        blk_dram = nc.inline_tensor(
            np.kron(np.eye(BP), np.ones((NCAND, 1))).astype(np.float32), name="blk"
        )
        blk32 = cpool.tile([BP * NCAND, BP], F32)
        nc.scalar.dma_start(out=blk32[:], in_=blk_dram[:])
        boff_dram = nc.inline_tensor(
            (np.arange(BP, dtype=np.float32) * M)[:, None], name="boff"
        )
        boff = cpool.tile([BP, 1], F32)
        nc.scalar.dma_start(out=boff[:], in_=boff_dram[:])
        # mask01[32b+hr, HB'*4? ] -- block-diagonal mask for the batched
        # stationary: 1 iff partition's batch == column's batch
        mask_dram = nc.inline_tensor(
            (np.arange(128)[:, None] // 32 == np.arange(4 * (H // 32))[None, :] % 4)
            .astype(np.float32), name="mask01"
        )
        mask01 = cpool.tile([128, 4 * (H // 32)], F32)
        nc.scalar.dma_start(out=mask01[:], in_=mask_dram[:])
        # R[hr, 32b+hr'] = (hr == hr'): replicates a [32, *] tile to 4 blocks
        rep_dram = nc.inline_tensor(
            (np.arange(128)[None, :] % 32 == np.arange(32)[:, None])
            .astype(np.float32), name="rep4"
        )
        rep4 = cpool.tile([32, 128], F32)
        nc.scalar.dma_start(out=rep4[:], in_=rep_dram[:])

        query_sb = wpool.tile([BP, H], F32)
        nc.scalar.dma_start(out=query_sb[:], in_=query[:])
        qb_sb = wpool.tile([1, H], F32)
        nc.scalar.dma_start(out=qb_sb[:], in_=qb[None, :])
        ow_sb = wpool.tile([128, HC * VOCAB], F32)
        nc.scalar.dma_start(out=ow_sb[:], in_=oww[:])
        ob_sb = wpool.tile([1, VOCAB], F32)
        nc.scalar.dma_start(out=ob_sb[:], in_=ob[None, :])

        # pre-warm the ACT exp table so the tail doesn't pay the ~2.7us load
        warm = cpool.tile([1, 1], F32)
        nc.vector.memset(warm[:], 0.0)
        nc.scalar.activation(
            out=warm[:], in_=warm[:], func=mybir.ActivationFunctionType.Exp,
        )

        # ---- prologue (all f32): qk = (k_w @ (q_w^T q + q_b)) ------------
        qT_ps = pp_sm.tile([128, HC * BP], F32, tag="tps")
        for c in range(HC):
            nc.tensor.transpose(
                out=qT_ps[:, c * BP:(c + 1) * BP],
                in_=query_sb[:, c * 128:(c + 1) * 128],
                identity=ident4[:],
            )
        qT_sb = wpool.tile([128, HC * BP], F32)
        nc.scalar.copy(out=qT_sb[:], in_=qT_ps[:])

        qa_ps = pp_acc.tile([BP, H], F32, tag="acc")
        nc.tensor.matmul(out=qa_ps[:], lhsT=ones1_bp[:], rhs=qb_sb[:], start=True, stop=False)
        for c in range(HC):
            nc.tensor.matmul(
                out=qa_ps[:],
                lhsT=qT_sb[:, c * BP:(c + 1) * BP],
                rhs=qw_sb[:, c * H:(c + 1) * H],
                start=False,
                stop=(c == HC - 1),
            )
        qa_sb = wpool.tile([BP, H], F32)
        nc.scalar.copy(out=qa_sb[:], in_=qa_ps[:])

        qaT_ps = pp_sm.tile([128, HC * BP], F32, tag="tps")
        for c in range(HC):
            nc.tensor.transpose(
                out=qaT_ps[:, c * BP:(c + 1) * BP],
                in_=qa_sb[:, c * 128:(c + 1) * 128],
                identity=ident4[:],
            )
        qaT_sb = wpool.tile([128, HC * BP], F32)
        nc.scalar.copy(out=qaT_sb[:], in_=qaT_ps[:])

        qk_ps = pp_acc.tile([BP, H], F32, tag="acc")
        for c in range(HC):
            nc.tensor.matmul(
                out=qk_ps[:],
                lhsT=qaT_sb[:, c * BP:(c + 1) * BP],
                rhs=kwt_sb[:, c * H:(c + 1) * H],
                start=(c == 0),
                stop=(c == HC - 1),
            )
        # unscaled copy (fp8 stationary source), scaled copy (exact rescore)
        qk_us = wpool.tile([BP, H], F32)
        nc.scalar.copy(out=qk_us[:], in_=qk_ps[:])
        qk_sb = wpool.tile([BP, H], F32)
        nc.scalar.mul(out=qk_sb[:], in_=qk_ps[:], mul=1.0 / math.sqrt(H))

        # block-diagonal fp8 stationary qkB8[32b+hr, 4hb+b'] =
        #   (b==b') * qk_us[b, 32hb+hr], built as mask01 * replicate(pattern)
        # pattern[hr, 4hb+b] = qk_us[b, 32hb+hr]: 32-col transposes of qk_us
        pattern_ps = pp_sm.tile([32, 4 * HB], F32, tag="pat")
        for hb in range(HB):
            nc.tensor.transpose(
                out=pattern_ps[:, hb * BP:(hb + 1) * BP],
                in_=qk_us[:, 32 * hb:32 * (hb + 1)],
                identity=ident4[:],
            )
        pattern = wpool.tile([32, 4 * HB], F32)
        nc.scalar.copy(out=pattern[:], in_=pattern_ps[:])
        rep_ps = pp_b.tile([128, 4 * HB], F32, tag="rep")
        nc.tensor.matmul(out=rep_ps[:], lhsT=rep4[:], rhs=pattern[:], start=True, stop=True)
        qkB8 = wpool.tile([128, 4 * HB], FP8)
        nc.vector.tensor_tensor(
            out=qkB8[:], in0=rep_ps[:], in1=mask01[:], op=mybir.AluOpType.mult
        )

        # qkb128[b*NCAND+k, :] = qk_sb[b, :] (scaled), for the exact rescore
        qkb_ps = pp_b.tile([BP * NCAND, H], F32, tag="qkb")
        nc.tensor.matmul(out=qkb_ps[:], lhsT=sel128[:], rhs=qk_sb[:], start=True, stop=True)
        qkb128 = wpool.tile([BP * NCAND, H], F32)
        nc.scalar.copy(out=qkb128[:], in_=qkb_ps[:])

        # ---- scoring on PE + per-chunk candidate extraction --------------
        # batched contraction: out[b, s] = sum_hb sum_hr
        #   qkB8[32b+hr, 4hb+b] * enc[b, s, 32hb+hr] -- all 4 rows valid.
        scores_row = rpool.tile([BP, M], F32, tag="scores")
        idxf = rpool.tile([BP, NCAND], F32, tag="idxf")
        for j in range(NCHUNK):
            ps = pp_s.tile([BP, S], F32, tag="score_ps")
            for hb in range(HB):
                nc.tensor.matmul(
                    out=ps[:],
                    lhsT=qkB8[:, 4 * hb:4 * (hb + 1)],
                    rhs=e_sbs[j][:, hb * S:(hb + 1) * S],
                    start=(hb == 0),
                    stop=(hb == HB - 1),
                )
            if j % 2 == 0:
                nc.vector.tensor_copy(scores_row[:, j * S:(j + 1) * S], ps[:])
            else:
                nc.scalar.copy(out=scores_row[:, j * S:(j + 1) * S], in_=ps[:])

            v8 = rpool.tile([BP, 8], F32, tag=f"v8_{j}")
            nc.vector.max(out=v8[:], in_=scores_row[:, j * S:(j + 1) * S])
            pos8 = rpool.tile([BP, 8], U32, tag=f"pos8_{j}")
            nc.vector.max_index(
                out=pos8[:], in_max=v8[:], in_values=scores_row[:, j * S:(j + 1) * S]
            )
            nc.vector.tensor_copy(idxf[:, j * 8:(j + 1) * 8], pos8[:])
            if j:
                nc.vector.tensor_scalar(
                    out=idxf[:, j * 8:(j + 1) * 8], in0=idxf[:, j * 8:(j + 1) * 8],
                    scalar1=float(j * S), scalar2=None, op0=mybir.AluOpType.add,
                )

        # ---- flat gather offsets -----------------------------------------
        nc.vector.tensor_scalar_add(idxf[:], idxf[:], boff[:, :1])
        idx_i32 = rpool.tile([BP, NCAND], I32, tag="idx_i32")
        nc.vector.tensor_copy(idx_i32[:], idxf[:])
        idx_col = rpool.tile([BP * NCAND, 1], I32, tag="idx_col")
        nc.scalar.dma_start(out=idx_col[:], in_=idx_i32[:])

        rows_sb = rpool.tile([BP * NCAND, H], F32, tag="rows_sb")
        nc.gpsimd.indirect_dma_start(
            out=rows_sb[:],
            out_offset=None,
            in_=enc32[:].rearrange("b m h -> (b m) h"),
            in_offset=IndirectOffsetOnAxis(ap=idx_col[:, 0:1], axis=0),
        )

        # ---- exact rescore ------------------------------------------------
        junk = rpool.tile([BP * NCAND, H], F32, tag="junk")
        s_col = rpool.tile([BP * NCAND, 1], F32, tag="s_col")
        nc.vector.scalar_tensor_tensor(
            out=junk[:], in0=rows_sb[:], scalar=1.0, in1=qkb128[:],
            op0=mybir.AluOpType.mult, op1=mybir.AluOpType.mult,
            accum_out=s_col[:],
        )
        s_row = rpool.tile([BP, NCAND], F32, tag="s_row")
        nc.scalar.dma_start(out=s_row[:], in_=s_col[:])

        # top-8 of the 32 exact scores; mask = selected
        vals = rpool.tile([BP, 8], F32, tag="vals")
        nc.vector.max(out=vals[:], in_=s_row[:])
        mr = rpool.tile([BP, NCAND], F32, tag="mr")
        nc.vector.match_replace(
            out=mr[:], in_to_replace=vals[:], in_values=s_row[:], imm_value=-1e30
        )
        m01 = rpool.tile([BP, NCAND], F32, tag="m01")
        nc.vector.tensor_scalar(
            out=m01[:], in0=mr[:], scalar1=-1e30, scalar2=None,
            op0=mybir.AluOpType.is_equal,
        )
        e_all = rpool.tile([BP, NCAND], F32, tag="e_all")
        nc.scalar.activation(
            out=e_all[:], in_=s_row[:], func=mybir.ActivationFunctionType.Exp,
        )
        e_sel = rpool.tile([BP, NCAND], F32, tag="e_sel")
        nc.vector.tensor_tensor(
            out=e_sel[:], in0=e_all[:], in1=m01[:], op=mybir.AluOpType.mult
        )
        zsum = rpool.tile([BP, 1], F32, tag="zsum")
        nc.vector.reduce_sum(out=zsum[:], in_=e_sel[:], axis=mybir.AxisListType.X)
        rz = rpool.tile([BP, 1], F32, tag="rz")
        nc.vector.reciprocal(out=rz[:], in_=zsum[:])
        probs = rpool.tile([BP, NCAND], F32, tag="probs")
        nc.vector.tensor_scalar_mul(probs[:], e_sel[:], rz[:, :1])
        p_col = rpool.tile([BP * NCAND, 1], F32, tag="p_col")
        nc.scalar.dma_start(out=p_col[:], in_=probs[:])

        # ---- retrieved^T = rows^T @ (blk*probs); logits -------------------
        wmat = rpool.tile([BP * NCAND, BP], F32, tag="wmat")
        nc.vector.tensor_scalar_mul(wmat[:], blk32[:], p_col[:, 0:1])
        retT_ps = pp_sm.tile([128, HC * BP], F32, tag="tps")
        for c in range(HC):
            nc.tensor.matmul(
                out=retT_ps[:, c * BP:(c + 1) * BP],
                lhsT=rows_sb[:, c * 128:(c + 1) * 128],
                rhs=wmat[:],
                start=True,
                stop=True,
            )
        xT_sb = rpool.tile([128, HC * BP], F32, tag="xT_sb")
        nc.vector.tensor_add(out=xT_sb[:], in0=retT_ps[:], in1=qT_sb[:])

        log_ps = pp_acc.tile([BP, VOCAB], F32, tag="acc")
        nc.tensor.matmul(out=log_ps[:], lhsT=ones1_bp[:], rhs=ob_sb[:], start=True, stop=False)
        for c in range(HC):
            nc.tensor.matmul(
                out=log_ps[:],
                lhsT=xT_sb[:, c * BP:(c + 1) * BP],
                rhs=ow_sb[:, c * VOCAB:(c + 1) * VOCAB],
                start=False,
                stop=(c == HC - 1),
            )
        log_sb = rpool.tile([BP, VOCAB], F32, tag="log_sb")
        nc.scalar.copy(out=log_sb[:], in_=log_ps[:])
        nc.sync.dma_start(out=logits[:], in_=log_sb[:])

    nc.compile()
    return nc


def get_nc():
    if "k" not in _CACHE:
        _CACHE["k"] = _build_kernel()
    return _CACHE["k"]


def _prepare_in_maps(enc_hidden, query_hidden, num_pairs, q_w, q_b, k_w, out_w, out_b):
    import ml_dtypes
    fp8 = ml_dtypes.float8_e4m3

    L = min(2 * int(num_pairs), T - 3)
    n_valid = max(0, min(L, M))
    start = max(0, L - M)

    qw32 = np.ascontiguousarray(
        np.asarray(q_w, dtype=np.float32)
        .reshape(HC, 128, H).transpose(1, 0, 2).reshape(128, HC * H)
    )
    kwt32 = np.ascontiguousarray(
        np.ascontiguousarray(np.asarray(k_w, dtype=np.float32).T)
        .reshape(HC, 128, H).transpose(1, 0, 2).reshape(128, HC * H)
    )
    qb = np.ascontiguousarray(q_b, dtype=np.float32)
    oww = np.ascontiguousarray(
        np.asarray(out_w, dtype=np.float32)
        .reshape(HC, 128, VOCAB).transpose(1, 0, 2).reshape(128, HC * VOCAB)
    )
    ob = np.ascontiguousarray(out_b, dtype=np.float32)

    in_maps = []
    for core in range(NCORES):
        b0 = core * BP
        sl = np.asarray(enc_hidden[b0:b0 + BP, start:start + n_valid, :], dtype=np.float32)
        if n_valid < M:
            pad = np.zeros((BP, M, H), dtype=np.float32)
            pad[:, :n_valid, :] = sl
            sl = pad
        else:
            sl = np.ascontiguousarray(sl)
        # encT8[j, 32b+hr, (hb, s)] = sl[b, j*S+s, 32*hb+hr]
        encT8 = np.ascontiguousarray(
            sl.reshape(BP, NCHUNK, S, H // 32, 32).transpose(1, 0, 4, 3, 2)
            .reshape(NCHUNK, 128, (H // 32) * S)
        ).astype(fp8)
        in_maps.append({
            "encT8": encT8,
            "enc32": sl,
            "query": np.ascontiguousarray(query_hidden[b0:b0 + BP, :], dtype=np.float32),
            "qw32": qw32,
            "kwt32": kwt32,
            "qb": qb,
            "oww": oww,
            "ob": ob,
        })
    return in_maps


def kernel(enc_hidden, query_hidden, num_pairs, q_w, q_b, k_w, k_b, out_w, out_b,
           **run_kwargs):
    """Full-input entry point: shards across 8 NeuronCores, returns (B, VOCAB).

    k_b is accepted (to match the reference signature) but unused: it shifts
    every attention score by the same per-batch constant, which affects
    neither the top-k selection nor the softmax probabilities.
    """
    enc_hidden = np.asarray(enc_hidden)
    query_hidden = np.asarray(query_hidden)
    nc = get_nc()
    in_maps = _prepare_in_maps(
        enc_hidden, query_hidden, num_pairs, q_w, q_b, k_w, out_w, out_b
    )
    res = run_bass_kernel_spmd(nc, in_maps, core_ids=list(range(NCORES)), **run_kwargs)
    out = np.concatenate([res.results[c]["logits"] for c in range(NCORES)], axis=0)
    kernel.last_results = res
    return out
